# revision 1
# baseline (speedup 1.0000x reference)
"""Trainium2 Bass kernel for the DinMod LSTM+CfC (NCP) recurrent network.

The graded execute path costs ~59us per emitted instruction (fully
serialized), so the design minimizes the instruction count of the T=512
scan: 27 compute + ~4 sync instructions per step (vs 33 + 5.6 for the
v1 baseline).

Per scan step:
  PE : gate inject (precomputed input projections -> one PSUM bank),
       layer-0 inject (own bank), whY_h/whY_F2/whAO_h/whAO_F2 (recurrent
       gate contractions vs the compact h-tile and the F2 tile), Cc
       (c' pair-sum), W0rec, P1, P2 (CfC preacts), D0/D1/D2 (each
       computes f2-f1 AND 0.5(f1+f2) in one matmul into a shared
       ping-pong PSUM bank so cross-step group-order waits subsume)
  ACT: YA (single Tanh for all 4 gates; sigmoid(x)=0.5(tanh(x/2)+1)
       with the 0.5 folded into host weights), Tc, F0, F1, F2
  DVE: S1, S2 (c' products), hl~ (=2*h_lstm), pt_l = tau_l*(f2-f1),
       h_l = 0.5*pt_l + 0.5(f1+f2) into the h-tile, out stt

h-tile [108, 8]: hl~@0:33, h0@64:82, h1@96:108 -- so the LSTM recurrence
and CfC layer-1/2 preacts each need ONE matmul against it; layer-2
output stays in the F2 quadrant tile (f1@0, f2@32, tau@64, pt@96),
contracted by the two wh_F2 matmuls.

Phase A: zin = Wcomb @ x with fc1 composed into all input projections
on the host; strided writes into Zin[97, T, 24].

Verifier constraints honored: every SBUF and PSUM access pattern starts
at partition 0/32/64/96 (max 128/32/64/32 rows); dual-SBUF vector
operands have equal base partitions (mixed SBUF/PSUM exempt);
partition-shifted writes are legal; PSUM accumulation groups are
bank-granular -- reads block until the group's stop covers every
started partition.
"""

import numpy as np

import concourse.bass as bass
import concourse.mybir as mybir
from concourse import bacc
from concourse.tile import TileContext
from concourse.tile_rust import add_dep_helper
from concourse.bass_utils import run_bass_kernel_spmd

IN_DIM, LATENT = 512, 256
INTER, COMMAND, MOTOR = 18, 12, 3
STATE = INTER + COMMAND + MOTOR  # 33
B, T_FULL, N_CORES = 64, 512, 8
BS = B // N_CORES  # 8
NF = T_FULL * BS   # 4096

F32 = mybir.dt.float32
AF = mybir.ActivationFunctionType
ALU = mybir.AluOpType

# ---------------------------------------------------------------------------
# Weight blob layout: every lhsT/bias lives at rows 0:k, cols off:off+m of a
# single [128, BW] tensor (one DMA, quadrant-legal base-0 slices).
# ---------------------------------------------------------------------------
_BLOCKS = [
    ("I97", 97, 97),
    ("whY_h", 108, 97), ("whY_F2", 99, 97),
    ("whAO_h", 108, 97), ("whAO_F2", 99, 97),
    ("Cc", 97, 33),
    ("W0rec", 33, 82), ("P1", 96, 76), ("P2", 108, 67),
    ("D0DS0", 50, 50), ("D1DS1", 44, 44), ("D2DS2", 35, 35),
    ("bias1", 76, 1), ("bias2", 67, 1),
    ("biasA", 97, 1), ("biasB", 97, 1), ("biasC", 82, 1),
] + [(f"pa{g}{k}", 128, 97 if g in "AB" else 82)
     for g in "ABC" for k in range(4)]

_OFFS = {}
_BW = 0
for _nm, _r, _c in _BLOCKS:
    _OFFS[_nm] = (_r, _c, _BW)
    _BW += _c


def prep_weights(inp):
    g = {k: np.asarray(v, np.float64) for k, v in inp.items()}
    Wf, bf = g["fc1_w"], g["fc1_b"]            # (256,512), (256,)
    wi, bi, wh = g["lstm_wi"], g["lstm_bi"], g["lstm_wh"]
    ia, ig, fg, og = (slice(0, 33), slice(33, 66),
                      slice(66, 99), slice(99, 132))

    blob = np.zeros((128, _BW), np.float64)

    def put(nm, m):
        r, c, off = _OFFS[nm]
        assert m.shape == (r, c), (nm, m.shape, (r, c))
        blob[0:r, off:off + c] = m

    put("I97", np.eye(97))

    # --- recurrent gate contractions ---
    # h-tile rows: hl~ @ 0:33 (=2*h_lstm), h0 @ 64:82, h1 @ 96:108
    # gate rows (packed): colgrp A: fg' 0:33, ig' 64:97 ; B: og' 0:33, ia 64:97
    # fg'/ig'/og' preacts carry a 0.5 factor (tanh-as-sigmoid); ia is raw.
    def wh_h(lo_sl, hi_sl, lo_s, hi_s):
        m = np.zeros((108, 97))
        for j in range(INTER):
            m[64 + j, 0:33] = lo_s * wh[lo_sl, j]
            m[64 + j, 64:97] = hi_s * wh[hi_sl, j]
        for j in range(COMMAND):
            m[96 + j, 0:33] = lo_s * wh[lo_sl, INTER + j]
            m[96 + j, 64:97] = hi_s * wh[hi_sl, INTER + j]
        return m

    def wh_f2(lo_sl, hi_sl, lo_s, hi_s):
        # h2 = 0.5*(f1+f2+pt2) (F2 tile rows f1@0:3 f2@32:35 pt2@96:99)
        m = np.zeros((99, 97))
        for j in range(MOTOR):
            for rr in (j, 32 + j, 96 + j):
                m[rr, 0:33] = 0.5 * lo_s * wh[lo_sl, 30 + j]
                m[rr, 64:97] = 0.5 * hi_s * wh[hi_sl, 30 + j]
        return m

    put("whY_h", wh_h(fg, ig, 0.5, 0.5))
    put("whY_F2", wh_f2(fg, ig, 0.5, 0.5))
    put("whAO_h", wh_h(og, ia, 0.5, 1.0))
    put("whAO_F2", wh_f2(og, ia, 0.5, 1.0))

    cc = np.zeros((97, 33))
    for j in range(33):
        cc[j, j] = 0.5
        cc[64 + j, j] = 0.5
    put("Cc", cc)

    # --- CfC layers: masked weights, ti = sig(wa x + ba + wb x + bb) ---
    w1m, w2m, wab, bab = [], [], [], []
    dims = [(LATENT, INTER), (INTER, COMMAND), (COMMAND, MOTOR)]
    for l, (p, k) in enumerate(dims):
        w1m.append(g[f"ff1w{l}"] * g[f"mask{l}"])
        w2m.append(g[f"ff2w{l}"] * g[f"mask{l}"])
        wab.append(g[f"taw{l}"] + g[f"tbw{l}"])
        bab.append(g[f"tab{l}"] + g[f"tbb{l}"])

    # W0rec: contracts htile[0:33] (= hl~); preact rows
    # f1@0:18 f2@32:50 tau@64:82; hl0_input = 0.5*hl~.
    m = np.zeros((33, 82))
    for j in range(INTER):
        m[j, 0:18] = 0.5 * w1m[0][:, LATENT + j]
        m[j, 32:50] = 0.5 * w2m[0][:, LATENT + j]
        m[j, 64:82] = 0.25 * wab[0][:, LATENT + j]
    put("W0rec", m)

    # P1: rhs htile[0:96]: rows 64:82 = h0 (exact); rows 18:30 = hl~[18:30].
    m = np.zeros((96, 76))
    for j in range(INTER):
        m[64 + j, 0:12] = w1m[1][:, j]
        m[64 + j, 32:44] = w2m[1][:, j]
        m[64 + j, 64:76] = 0.5 * wab[1][:, j]
    for j in range(COMMAND):
        m[18 + j, 0:12] = 0.5 * w1m[1][:, INTER + j]
        m[18 + j, 32:44] = 0.5 * w2m[1][:, INTER + j]
        m[18 + j, 64:76] = 0.25 * wab[1][:, INTER + j]
    put("P1", m)

    # P2: rhs htile[0:108]: rows 96:108 = h1; rows 30:33 = hl~[30:33].
    m = np.zeros((108, 67))
    for j in range(COMMAND):
        m[96 + j, 0:3] = w1m[2][:, j]
        m[96 + j, 32:35] = w2m[2][:, j]
        m[96 + j, 64:67] = 0.5 * wab[2][:, j]
    for j in range(MOTOR):
        m[30 + j, 0:3] = 0.5 * w1m[2][:, COMMAND + j]
        m[30 + j, 32:35] = 0.5 * w2m[2][:, COMMAND + j]
        m[30 + j, 64:67] = 0.25 * wab[2][:, COMMAND + j]
    put("P2", m)

    for nm, k in (("D0DS0", INTER), ("D1DS1", COMMAND), ("D2DS2", MOTOR)):
        m = np.zeros((32 + k, 32 + k))
        for j in range(k):
            m[j, j] = -1.0          # D = f2 - f1 @ rows 0:k
            m[32 + j, j] = 1.0
            m[j, 32 + j] = 0.5      # DS = 0.5(f1+f2) @ rows 32:32+k
            m[32 + j, 32 + j] = 0.5
        put(nm, m)

    bs1 = np.zeros((76, 1))
    bs1[0:12, 0] = g["ff1b1"]
    bs1[32:44, 0] = g["ff2b1"]
    bs1[64:76, 0] = 0.5 * bab[1]
    put("bias1", bs1)
    bs2 = np.zeros((67, 1))
    bs2[0:3, 0] = g["ff1b2"]
    bs2[32:35, 0] = g["ff2b2"]
    bs2[64:67, 0] = 0.5 * bab[2]
    put("bias2", bs2)

    # --- phase A composed input projections: zin = Wg @ x + bg ---
    WA = np.zeros((97, IN_DIM)); bA = np.zeros((97, 1))
    WA[0:33] = 0.5 * (wi[fg] @ Wf)
    bA[0:33, 0] = 0.5 * (wi[fg] @ bf + bi[fg] + 1.0)
    WA[64:97] = 0.5 * (wi[ig] @ Wf)
    bA[64:97, 0] = 0.5 * (wi[ig] @ bf + bi[ig])
    WB = np.zeros((97, IN_DIM)); bB = np.zeros((97, 1))
    WB[0:33] = 0.5 * (wi[og] @ Wf)
    bB[0:33, 0] = 0.5 * (wi[og] @ bf + bi[og])
    WB[64:97] = wi[ia] @ Wf
    bB[64:97, 0] = wi[ia] @ bf + bi[ia]
    WC = np.zeros((82, IN_DIM)); bC = np.zeros((82, 1))
    WC[0:18] = w1m[0][:, 0:LATENT] @ Wf
    bC[0:18, 0] = w1m[0][:, 0:LATENT] @ bf + g["ff1b0"]
    WC[32:50] = w2m[0][:, 0:LATENT] @ Wf
    bC[32:50, 0] = w2m[0][:, 0:LATENT] @ bf + g["ff2b0"]
    WC[64:82] = 0.5 * (wab[0][:, 0:LATENT] @ Wf)
    bC[64:82, 0] = 0.5 * (wab[0][:, 0:LATENT] @ bf + bab[0])
    put("biasA", bA)
    put("biasB", bB)
    put("biasC", bC)
    for gname, W in (("A", WA), ("B", WB), ("C", WC)):
        for k in range(4):
            put(f"pa{gname}{k}",
                np.ascontiguousarray(W[:, 128 * k:128 * (k + 1)].T))

    return {"wblob": blob.astype(np.float32)}


def build_program(T=T_FULL, opts=()):
    opts = set(opts)
    nf = T * BS
    nch = nf // 512
    CH = 512
    CHS = CH // BS  # steps per phase-A chunk

    nc = bacc.Bacc("TRN2")
    xt_d = nc.dram_tensor("xt", [IN_DIM, nf], F32, kind="ExternalInput")
    wb_d = nc.dram_tensor("wblob", [128, _BW], F32, kind="ExternalInput")
    out_d = nc.dram_tensor("out", [MOTOR, nf], F32, kind="ExternalOutput")

    with TileContext(nc) as tc:
        with tc.tile_pool(name="wpool", bufs=1) as wp, \
             tc.tile_pool(name="data", bufs=1) as dp:
            wb = wp.tile([128, _BW], F32, tag="wb")
            nc.sync.dma_start(out=wb, in_=wb_d[:, :])

            def W(nm):
                r, c, off = _OFFS[nm]
                return wb[0:r, off:off + c]

            xt_sb = dp.tile([128, 4, nf], F32)
            nc.sync.dma_start(out=xt_sb,
                              in_=xt_d.rearrange("(c p) n -> p c n", p=128))

            zin = dp.tile([97, T, 24], F32)
            out_sb = dp.tile([MOTOR, T, BS], F32)

            # ---- Phase A: zin = Wg @ x + bg, strided into zin groups ----
            with tc.tile_pool(name="pa", bufs=1, space="PSUM") as pa:
                for gname, rows, c0, c1 in (("A", 97, 0, 8), ("B", 97, 8, 16),
                                            ("C", 82, 16, 24)):
                    for n in range(nch):
                        ps = pa.tile([rows, CH], F32)
                        for k in range(4):
                            nc.tensor.matmul(
                                ps, W(f"pa{gname}{k}")[:, 0:rows],
                                xt_sb[:, k, n * CH:(n + 1) * CH],
                                start=(k == 0), stop=(k == 3))
                        nc.scalar.activation(
                            zin[0:rows, n * CHS:(n + 1) * CHS, c0:c1], ps,
                            AF.Identity, bias=W(f"bias{gname}")[:, 0:1])

            # ---- Phase B: the scan ----
            with tc.tile_pool(name="st", bufs=1) as stp, \
                 tc.tile_pool(name="pP", bufs=1, space="PSUM") as pP, \
                 tc.tile_pool(name="pC", bufs=1, space="PSUM") as pC, \
                 tc.tile_pool(name="pC2", bufs=1, space="PSUM") as pC2, \
                 tc.tile_pool(name="p1", bufs=1, space="PSUM") as p1p, \
                 tc.tile_pool(name="p2", bufs=1, space="PSUM") as p2p, \
                 tc.tile_pool(name="pDe", bufs=1, space="PSUM") as pDep, \
                 tc.tile_pool(name="pDo", bufs=1, space="PSUM") as pDop, \
                 tc.tile_pool(name="pG", bufs=1, space="PSUM") as pGp:

                htile = stp.tile([108, BS], F32, tag="htile")
                F0 = stp.tile([82, BS], F32, tag="F0")
                F1 = stp.tile([76, BS], F32, tag="F1")
                F2 = stp.tile([99, BS], F32, tag="F2")
                YA = stp.tile([97, 2, BS], F32, tag="YA")
                Tc = stp.tile([33, BS], F32, tag="Tc")
                S = stp.tile([97, BS], F32, tag="S")
                G = stp.tile([18, BS], F32, tag="G")
                PcA = pC.tile([33, BS], F32, tag="PcA")
                PcB = pC2.tile([33, BS], F32, tag="PcB")
                for t_ in (htile, F2, S):
                    nc.vector.memset(t_, 0.0)
                nc.vector.memset(PcA, 0.0)
                nc.vector.memset(PcB, 0.0)

                scan_reps = 1
                for o in opts:
                    if isinstance(o, str) and o.startswith("reps"):
                        scan_reps = int(o[4:])

                prev_f2act = None
                prev_out = None
                for rep in range(scan_reps):
                    for t in range(T):
                        Pcw = PcA if t % 2 == 0 else PcB
                        Pcr = PcB if t % 2 == 0 else PcA
                        Db = (pDep if t % 2 == 0 else pDop).tile(
                            [50, 3, BS], F32, tag="Db")
                        P = pP.tile([97, 16], F32)
                        Pg = pGp.tile([82, BS], F32, tag="Pg")
                        minj = nc.tensor.matmul(P, W("I97"), zin[0:97, t, 0:16],
                                                start=True, stop=False)
                        if prev_f2act is not None:
                            add_dep_helper(minj.ins, prev_f2act.ins, sync=True,
                                           reason="wait budget: ACT watermark")
                        nc.tensor.matmul(Pg, W("I97")[0:82, 0:82],
                                         zin[0:82, t, 16:24],
                                         start=True, stop=False)
                        nc.tensor.matmul(P[0:97, 0:8], W("whY_h"),
                                         htile[0:108, :], start=False, stop=False)
                        nc.tensor.matmul(P[0:97, 0:8], W("whY_F2"),
                                         F2[0:99, :], start=False, stop=False)
                        nc.tensor.matmul(P[0:97, 8:16], W("whAO_h"),
                                         htile[0:108, :], start=False, stop=False)
                        mao = nc.tensor.matmul(P[0:97, 8:16], W("whAO_F2"),
                                               F2[0:99, :], start=False,
                                               stop=True)

                        aya = nc.scalar.activation(YA[0:97, :, :],
                                                   P[0:97, 0:16], AF.Tanh)
                        if prev_out is not None:
                            add_dep_helper(aya.ins, prev_out.ins, sync=True,
                                           reason="wait budget: DVE watermark")
                        if prev_f2act is not None:
                            add_dep_helper(aya.ins, prev_f2act.ins, sync=True,
                                           reason="wait budget: ACT watermark")
                        # S1 = (tau_fg+1)*c_prev ; S2 = (tau_ig+1)*tanh(ia)
                        nc.vector.scalar_tensor_tensor(
                            S[0:33, :], YA[0:33, 0, :], 1.0, Pcr,
                            ALU.add, ALU.mult)
                        ms2 = nc.vector.scalar_tensor_tensor(
                            S[64:97, :], YA[64:97, 0, :], 1.0, YA[64:97, 1, :],
                            ALU.add, ALU.mult)
                        add_dep_helper(ms2.ins, mao.ins, sync=True,
                                       reason="wait budget: PE watermark")
                        nc.tensor.matmul(Pcw, W("Cc"), S[0:97, :],
                                         start=True, stop=True)
                        nc.scalar.activation(Tc, Pcw, AF.Tanh)
                        # hl~ = (tau_og+1)*tanh(c') = 2*h_lstm -> htile[0:33]
                        nc.vector.scalar_tensor_tensor(
                            htile[0:33, :], YA[0:33, 1, :], 1.0, Tc,
                            ALU.add, ALU.mult)

                        # CfC layer 0: preact -> tanh -> D=f2-f1,
                        # q=(tau+1)*D, h0 = f1 + 0.5*q
                        nc.tensor.matmul(Pg, W("W0rec"), htile[0:33, :],
                                         start=False, stop=True)
                        nc.scalar.activation(F0[0:82, :], Pg, AF.Tanh)
                        nc.tensor.matmul(Db[0:50, 0, :], W("D0DS0"),
                                         F0[0:50, :], start=True, stop=True)
                        nc.vector.tensor_mul(G[0:18, :], F0[64:82, :],
                                             Db[0:18, 0, :])
                        nc.vector.scalar_tensor_tensor(
                            htile[64:82, :], G[0:18, :], 0.5, Db[32:50, 0, :],
                            ALU.mult, ALU.add)

                        # CfC layer 1
                        P1 = p1p.tile([76, BS], F32)
                        nc.tensor.matmul(P1, W("P1"), htile[0:96, :],
                                         start=True, stop=True)
                        nc.scalar.activation(F1[0:76, :], P1, AF.Tanh,
                                             bias=W("bias1")[:, 0:1])
                        nc.tensor.matmul(Db[0:44, 1, :], W("D1DS1"),
                                         F1[0:44, :], start=True, stop=True)
                        nc.vector.tensor_mul(G[0:12, :], F1[64:76, :],
                                             Db[0:12, 1, :])
                        nc.vector.scalar_tensor_tensor(
                            htile[96:108, :], G[0:12, :], 0.5, Db[32:44, 1, :],
                            ALU.mult, ALU.add)

                        # CfC layer 2: q2 -> F2[96:99]; out = f1 + 0.5*q2
                        P2 = p2p.tile([67, BS], F32)
                        nc.tensor.matmul(P2, W("P2"), htile[0:108, :],
                                         start=True, stop=True)
                        prev_f2act = nc.scalar.activation(
                            F2[0:67, :], P2, AF.Tanh, bias=W("bias2")[:, 0:1])
                        nc.tensor.matmul(Db[0:35, 2, :], W("D2DS2"),
                                         F2[0:35, :], start=True, stop=True)
                        nc.vector.tensor_mul(F2[96:99, :], F2[64:67, :],
                                             Db[0:3, 2, :])
                        prev_out = nc.vector.scalar_tensor_tensor(
                            out_sb[:, t, :], F2[96:99, :], 0.5, Db[32:35, 2, :],
                            ALU.mult, ALU.add)

            nc.sync.dma_start(out=out_d[:, :],
                              in_=out_sb.rearrange("m t b -> m (t b)"))
    nc.compile()
    return nc


def _reduce_sync_deps(nc):
    """Transitive reduction of sync dependencies.

    A sync dep i->d guarantees d COMPLETED before i issues. Engine queues
    issue in order and complete in order (each instruction bumps its
    engine's semaphore by 1), so:
      - i inherits all guarantees of its same-engine predecessor (the
        queue stalls on its waits before i issues), and
      - d completed implies every earlier instruction on d's engine
        completed, with all of THEIR guarantees.
    Any dep already implied this way is removed, so the bacc lowering
    emits at most one semaphore wait per instruction and almost no
    InstEventSemaphore splits. Deps involving DMA/SP instructions are
    left untouched (their completion order is per-queue, not per-engine).
    """
    insts = [ins for bb in nc.m.functions[0].blocks for ins in bb.instructions]
    NENG = 8
    eng_ids = {}
    info = {}       # name -> (eng_id, engine_idx, global_idx)
    safe_eng = {}   # eng_id -> True if completion-in-order reasoning applies
    counters = {}
    for gi, ins in enumerate(insts):
        e = str(ins.engine)
        if e not in eng_ids:
            eng_ids[e] = len(eng_ids)
            safe_eng[eng_ids[e]] = e in (
                "EngineType.PE", "EngineType.DVE", "EngineType.Activation",
                "EngineType.Pool")
        eid = eng_ids[e]
        k = counters.get(eid, 0)
        counters[eid] = k + 1
        dma = type(ins).__name__ in ("InstDMACopy", "InstTensorLoad",
                                     "InstTensorSave")
        info[ins.name] = (eid, k, gi, dma)

    NEG = -1
    # cvc_max[e] = running elementwise max of completed-VCs on engine e
    cvc_max = [[NEG] * NENG for _ in range(NENG)]
    cvc = {}        # name -> completed-VC (list)
    last_on_eng = [None] * NENG  # name of previous instruction on engine
    removed = 0
    for ins in insts:
        nm = ins.name
        eid, k, gi, dma = info[nm]
        # issue-VC: inherited from same-engine predecessor's issue-VC...
        # we conservatively use predecessor's *completed* prefix guarantees:
        # pred's issue-VC == what pred's waits guaranteed; i issues after
        # those same waits were satisfied.
        vc = list(cvc[last_on_eng[eid]]) if last_on_eng[eid] else [NEG] * NENG
        if last_on_eng[eid]:
            # pred's completion itself is NOT guaranteed at i's issue
            pvc = cvc[last_on_eng[eid]]
            peid, pk = info[last_on_eng[eid]][0], info[last_on_eng[eid]][1]
            vc = list(pvc)
            if vc[peid] >= pk:   # pred's own completion not implied
                vc[peid] = pk - 1
        deps = list(ins.sync_dependency_names())
        dep_info = []
        for d in deps:
            di = info.get(d)
            if di is None:
                continue
            dep_info.append((d, di))
        # keep deps strongest-first (later global index first)
        dep_info.sort(key=lambda x: -x[1][2])
        for d, (deid, dk, dgi, ddma) in dep_info:
            if dma or ddma or not safe_eng.get(deid, False):
                # never touch DMA-related deps; merge their guarantees
                dvc = cvc.get(d)
                if dvc:
                    for j in range(NENG):
                        if dvc[j] > vc[j]:
                            vc[j] = dvc[j]
                    if dk > vc[deid]:
                        vc[deid] = dk
                continue
            if vc[deid] >= dk:
                if ins.try_remove_dependency(d):
                    removed += 1
                    continue
            dvc = cvc.get(d, [NEG] * NENG)
            for j in range(NENG):
                if dvc[j] > vc[j]:
                    vc[j] = dvc[j]
            if dk > vc[deid]:
                vc[deid] = dk
        # completed-VC of this instruction: issue-VC + own completion,
        # folded with everything earlier on this engine
        cm = cvc_max[eid]
        for j in range(NENG):
            if cm[j] > vc[j]:
                vc[j] = cm[j]
        if k > vc[eid]:
            vc[eid] = k
        cvc[nm] = vc
        cvc_max[eid] = list(vc)
        last_on_eng[eid] = nm
    import sys
    print(f"_reduce_sync_deps: removed {removed} redundant deps",
          file=sys.stderr)


def host_prep(inputs, T=T_FULL):
    x = np.asarray(inputs["x"], np.float32)
    w = prep_weights(inputs)
    in_maps = []
    for i in range(N_CORES):
        xs = x[i * BS:(i + 1) * BS, :T, :]                  # (BS, T, 512)
        xt = np.ascontiguousarray(
            xs.transpose(2, 1, 0).reshape(IN_DIM, T * BS))
        m = {"xt": xt}
        m.update(w)
        in_maps.append(m)
    return in_maps


def gather_output(results, T=T_FULL):
    outs = []
    for i in range(N_CORES):
        o = np.asarray(results[i]["out"])                   # (3, T*BS)
        outs.append(o.reshape(MOTOR, T, BS).transpose(2, 1, 0))
    return np.concatenate(outs, axis=0)


_PROGRAM_CACHE = {}


def kernel(**inputs):
    T = T_FULL
    if T not in _PROGRAM_CACHE:
        _PROGRAM_CACHE[T] = build_program(T)
    nc = _PROGRAM_CACHE[T]
    in_maps = host_prep(inputs, T)
    res = run_bass_kernel_spmd(nc, in_maps, list(range(N_CORES)))
    return gather_output(res.results, T)



# revision 5
# speedup vs baseline: 1.8655x; 1.8655x over previous
"""Trainium2 Bass kernel for the DinMod LSTM+CfC (NCP) recurrent network.

Parallel-in-time Picard iteration. The graded execute path costs ~45us
per emitted instruction (fully serialized, size-independent), so instead
of an exact T=512 sequential scan (~16k instructions), we iterate the
whole trajectory: guess h[t]=0, then each sweep recomputes all T steps
with BATCHED instructions (matmuls/ACT/DVE over 4 sequences x 512 steps
at once). The LSTM c-recurrence is linear given the gates, so ONE
tensor_tensor_scan instruction solves it exactly along the time axis per
sweep. The step map is a strong contraction (~10x error reduction per
sweep, measured on the actual weights): 6 sweeps reach ~3e-6 relative
error in fp64 (tolerance is 2e-2).

Per core: 8 sequences (batch elems), processed as 2 independent halves
of 4 (SBUF budget). Column layout per half: 4 segments of 513 columns
(1 zero pad + 512 steps); col(s, t) = s*513 + 1 + t. Trajectory tiles
read at offset 0 give h[t-1] (shifted), offset 1 gives h[t]; the pad
column doubles as the zero initial state and as the scan reset (the
sfg multiplier and S2 addend are 0 there, so the running c state resets
across segment boundaries in the single flattened scan).

Per sweep per half (~65 instructions):
  gates: 4 segs x (inject zinA | whA@h | inject zinB | whB@h) = 16 mm
         -> sigmoid(fg+1)/sigmoid(og) (1 ACT over both groups),
            sigmoid(ig), tanh(ia) [+1 folded into the zinA bias]
  c:     S2 = sig*tia (DVE), c = tensor_tensor_scan(sfg, S2),
         tanh(c) (ACT), h_lstm = tc*sog (DVE)
  CfC l: 2 mm/seg (inject or h0/h1-part + recurrent part), 3 ACT
         (tanh f1, tanh f2, sigmoid ti), 3 DVE (D=f2-f1, G=ti*D,
         h_l = f1+G -> written time-shifted into the H tile)

Verifier constraints honored: all access patterns start at partition
0/32/64/96; dual-SBUF vector operands have equal base partitions
(mixed SBUF/PSUM exempt); PSUM accumulation groups are start/stop pairs
within one bank.
"""

import numpy as np

import concourse.bass as bass
import concourse.mybir as mybir
from concourse import bacc
from concourse.tile import TileContext
from concourse.bass_utils import run_bass_kernel_spmd

IN_DIM, LATENT = 512, 256
INTER, COMMAND, MOTOR = 18, 12, 3
STATE = INTER + COMMAND + MOTOR  # 33
B, T_FULL, N_CORES = 64, 512, 8
BS = B // N_CORES  # 8 sequences per core
HB = 4             # sequences per half
SEG = T_FULL + 1   # 513 padded columns per sequence
NH = HB * SEG      # 2052 columns per half
NSWEEPS = 6

F32 = mybir.dt.float32
AF = mybir.ActivationFunctionType
ALU = mybir.AluOpType

# ---------------------------------------------------------------------------
# Weight blob: every lhsT/bias lives at rows 0:r, cols off:off+c of a single
# [128, BW] tensor (one DMA; base-0 quadrant-legal slices).
# ---------------------------------------------------------------------------
# (name, rows, cols, base_row): base_row shifts the block down so the
# lhsT slice's base partition matches its rhs (matmul requires equality).
_BLOCKS = [
    ("I97", 97, 97, 0), ("I82", 82, 82, 0),
    ("whA", 67, 97, 0), ("whB", 67, 97, 0),
    ("W0r", 18, 82, 0),
    ("P1h0", 18, 76, 0), ("P1h", 33, 76, 0),
    ("P2h1", 12, 67, 32), ("P2h", 33, 67, 0),
    ("bA", 97, 1, 0), ("bB", 97, 1, 0), ("bC", 82, 1, 0),
    ("b1", 76, 1, 0), ("b2", 67, 1, 0),
] + [(f"pa{g}{k}", 128, 97 if g in "AB" else 82, 0)
     for g in "ABC" for k in range(4)]

_OFFS = {}
_BW = 0
for _nm, _r, _c, _b in _BLOCKS:
    _OFFS[_nm] = (_r, _c, _BW, _b)
    _BW += _c

# HT (h trajectory) row layout: h0@0:18, h1@32:44, h2@64:67 (quadrant-legal
# write starts for the three per-layer h writers); unused rows stay 0.
_HTROW = [j for j in range(18)] + [32 + j for j in range(12)] + [64 + j for j in range(3)]


def prep_weights(inp):
    g = {k: np.asarray(v, np.float64) for k, v in inp.items()}
    Wf, bf = g["fc1_w"], g["fc1_b"]            # (256,512), (256,)
    wi, bi, wh = g["lstm_wi"], g["lstm_bi"], g["lstm_wh"]
    ia, ig, fg, og = (slice(0, 33), slice(33, 66),
                      slice(66, 99), slice(99, 132))

    blob = np.zeros((128, _BW), np.float64)

    def put(nm, m):
        r, c, off, b = _OFFS[nm]
        assert m.shape == (r, c), (nm, m.shape, (r, c))
        blob[b:b + r, off:off + c] = m

    put("I97", np.eye(97))
    put("I82", np.eye(82))

    # recurrent gate contractions: out rows (grp A) fg@0:33, ig@64:97;
    # (grp B) og@0:33, ia@64:97. Contract dim = HT rows (h comp j at _HTROW[j]).
    def wh_block(lo_sl, hi_sl):
        m = np.zeros((67, 97))
        for j in range(STATE):
            r = _HTROW[j]
            m[r, 0:33] = wh[lo_sl, j]
            m[r, 64:97] = wh[hi_sl, j]
        return m

    put("whA", wh_block(fg, ig))
    put("whB", wh_block(og, ia))

    # CfC layer weights (masked), ti = sigmoid((ta+tb) @ xc + (tab+tbb))
    w1m, w2m, wab, bab = [], [], [], []
    for l in range(3):
        w1m.append(g[f"ff1w{l}"] * g[f"mask{l}"])
        w2m.append(g[f"ff2w{l}"] * g[f"mask{l}"])
        wab.append(g[f"taw{l}"] + g[f"tbw{l}"])
        bab.append(g[f"tab{l}"] + g[f"tbb{l}"])

    # layer 0 recurrent part: contracts HL[0:18] (= hs0); out rows
    # f1@0:18, f2@32:50, ti@64:82
    m = np.zeros((18, 82))
    for j in range(INTER):
        m[j, 0:18] = w1m[0][:, LATENT + j]
        m[j, 32:50] = w2m[0][:, LATENT + j]
        m[j, 64:82] = wab[0][:, LATENT + j]
    put("W0r", m)

    # layer 1: xc = [h0(18), hs1(12)]; out rows f1@0:12, f2@32:44, ti@64:76
    m = np.zeros((18, 76))
    for j in range(INTER):
        m[j, 0:12] = w1m[1][:, j]
        m[j, 32:44] = w2m[1][:, j]
        m[j, 64:76] = wab[1][:, j]
    put("P1h0", m)
    m = np.zeros((33, 76))
    for j in range(COMMAND):
        m[INTER + j, 0:12] = w1m[1][:, INTER + j]
        m[INTER + j, 32:44] = w2m[1][:, INTER + j]
        m[INTER + j, 64:76] = wab[1][:, INTER + j]
    put("P1h", m)
    bs1 = np.zeros((76, 1))
    bs1[0:12, 0] = g["ff1b1"]
    bs1[32:44, 0] = g["ff2b1"]
    bs1[64:76, 0] = bab[1]
    put("b1", bs1)

    # layer 2: xc = [h1(12), hs2(3)]; out rows f1@0:3, f2@32:35, ti@64:67
    m = np.zeros((12, 67))
    for j in range(COMMAND):
        m[j, 0:3] = w1m[2][:, j]
        m[j, 32:35] = w2m[2][:, j]
        m[j, 64:67] = wab[2][:, j]
    put("P2h1", m)
    m = np.zeros((33, 67))
    for j in range(MOTOR):
        m[30 + j, 0:3] = w1m[2][:, COMMAND + j]
        m[30 + j, 32:35] = w2m[2][:, COMMAND + j]
        m[30 + j, 64:67] = wab[2][:, COMMAND + j]
    put("P2h", m)
    bs2 = np.zeros((67, 1))
    bs2[0:3, 0] = g["ff1b2"]
    bs2[32:35, 0] = g["ff2b2"]
    bs2[64:67, 0] = bab[2]
    put("b2", bs2)

    # phase A composed input projections (fc1 folded in); +1.0 on the fg
    # bias so the sigmoid ACT needs no extra bias.
    WA = np.zeros((97, IN_DIM)); bA = np.zeros((97, 1))
    WA[0:33] = wi[fg] @ Wf
    bA[0:33, 0] = wi[fg] @ bf + bi[fg] + 1.0
    WA[64:97] = wi[ig] @ Wf
    bA[64:97, 0] = wi[ig] @ bf + bi[ig]
    WB = np.zeros((97, IN_DIM)); bB = np.zeros((97, 1))
    WB[0:33] = wi[og] @ Wf
    bB[0:33, 0] = wi[og] @ bf + bi[og]
    WB[64:97] = wi[ia] @ Wf
    bB[64:97, 0] = wi[ia] @ bf + bi[ia]
    WC = np.zeros((82, IN_DIM)); bC = np.zeros((82, 1))
    WC[0:18] = w1m[0][:, 0:LATENT] @ Wf
    bC[0:18, 0] = w1m[0][:, 0:LATENT] @ bf + g["ff1b0"]
    WC[32:50] = w2m[0][:, 0:LATENT] @ Wf
    bC[32:50, 0] = w2m[0][:, 0:LATENT] @ bf + g["ff2b0"]
    WC[64:82] = wab[0][:, 0:LATENT] @ Wf
    bC[64:82, 0] = wab[0][:, 0:LATENT] @ bf + bab[0]
    put("bA", bA)
    put("bB", bB)
    put("bC", bC)
    for gname, W in (("A", WA), ("B", WB), ("C", WC)):
        for k in range(4):
            put(f"pa{gname}{k}",
                np.ascontiguousarray(W[:, 128 * k:128 * (k + 1)].T))

    return {"wblob": blob.astype(np.float32)}


def build_program(T=T_FULL, opts=()):
    assert T == T_FULL
    opts = set(opts)
    sweep_reps = 1
    for o in opts:
        if isinstance(o, str) and o.startswith("reps"):
            sweep_reps = int(o[4:])

    nc = bacc.Bacc("TRN2")
    xt_d = nc.dram_tensor("xt", [IN_DIM, BS * T], F32, kind="ExternalInput")
    wb_d = nc.dram_tensor("wblob", [128, _BW], F32, kind="ExternalInput")
    out_d = nc.dram_tensor("out", [MOTOR, BS, T], F32, kind="ExternalOutput")

    with TileContext(nc) as tc:
        with tc.tile_pool(name="wpool", bufs=1) as wp, \
             tc.tile_pool(name="data", bufs=1) as dp:
            wb = wp.tile([128, _BW], F32, name="wb")
            nc.sync.dma_start(out=wb, in_=wb_d[:, :])

            def W(nm):
                r, c, off, b = _OFFS[nm]
                return wb[b:b + r, off:off + c]

            # persistent per-half tiles (reused by both halves; only the
            # pad columns must stay zero, and nothing ever writes them)
            zinA = dp.tile([97, HB, T], F32, name="zinA")
            zinB = dp.tile([97, HB, T], F32, name="zinB")
            zinC = dp.tile([82, HB, T], F32, name="zinC")
            SGt = dp.tile([33, 2, HB, SEG], F32, name="SGt")  # sfg | sog
            SIG = dp.tile([33, HB, T], F32, name="SIG")
            TIA = dp.tile([33, HB, T], F32, name="TIA")
            S2T = dp.tile([33, HB, SEG], F32, name="S2T")
            CT = dp.tile([33, HB, SEG], F32, name="CT")
            TC = dp.tile([33, HB, T], F32, name="TC")
            HT = dp.tile([67, HB, SEG], F32, name="HT")
            HL = dp.tile([33, HB, T], F32, name="HL")
            Ff1 = dp.tile([18, HB, T], F32, name="Ff1")
            Ff2 = dp.tile([18, HB, T], F32, name="Ff2")
            Fti = dp.tile([18, HB, T], F32, name="Fti")
            Dg = dp.tile([18, HB, T], F32, name="Dg")
            Gg = dp.tile([18, HB, T], F32, name="Gg")

            nc.vector.memset(SGt, 0.0)
            nc.vector.memset(S2T, 0.0)
            nc.vector.memset(HT, 0.0)

            sfg_flat = SGt.rearrange("p g s c -> p g (s c)")[0:33, 0, 0:NH]
            s2_flat = S2T.rearrange("p s c -> p (s c)")
            ct_flat = CT.rearrange("p s c -> p (s c)")

            xt_r = xt_d.rearrange("(c p) n -> p c n", p=128)

            for half in range(2):
                hc0 = half * HB * T  # first input column of this half
                with tc.tile_pool(name="xp", bufs=1) as xp:
                    xt_sb = xp.tile([128, 4, HB * T], F32, name="xt_sb")
                    nc.sync.dma_start(
                        out=xt_sb, in_=xt_r[:, :, hc0:hc0 + HB * T])

                    # ---- phase A: input projections -> zinA/zinB/zinC ----
                    with tc.tile_pool(name="pa", bufs=1, space="PSUM") as pa:
                        pg = pa.tile([97, 2, HB, T], F32, name="pg")
                        for gi, gname in ((0, "A"), (1, "B")):
                            for s in range(HB):
                                for k in range(4):
                                    nc.tensor.matmul(
                                        pg[0:97, gi, s, :],
                                        W(f"pa{gname}{k}")[:, 0:97],
                                        xt_sb[:, k, s * T:(s + 1) * T],
                                        start=(k == 0), stop=(k == 3))
                        nc.scalar.activation(zinA, pg[0:97, 0, :, :],
                                             AF.Identity, bias=W("bA")[:, 0:1])
                        nc.scalar.activation(zinB, pg[0:97, 1, :, :],
                                             AF.Identity, bias=W("bB")[:, 0:1])
                    with tc.tile_pool(name="pc", bufs=1, space="PSUM") as pc:
                        pgc = pc.tile([82, HB, T], F32, name="pgc")
                        for s in range(HB):
                            for k in range(4):
                                nc.tensor.matmul(
                                    pgc[0:82, s, :], W(f"paC{k}")[:, 0:82],
                                    xt_sb[:, k, s * T:(s + 1) * T],
                                    start=(k == 0), stop=(k == 3))
                        nc.scalar.activation(zinC, pgc, AF.Identity,
                                             bias=W("bC")[:, 0:1])

                # ---- Picard sweeps ----
                for sw in range(NSWEEPS * sweep_reps):
                    # gates
                    with tc.tile_pool(name="pq", bufs=1, space="PSUM") as pq:
                        gt = pq.tile([97, 2, HB, T], F32, name="gt")
                        for s in range(HB):
                            nc.tensor.matmul(gt[0:97, 0, s, :], W("I97"),
                                             zinA[0:97, s, :],
                                             start=True, stop=False)
                            nc.tensor.matmul(gt[0:97, 0, s, :], W("whA"),
                                             HT[0:67, s, 0:T],
                                             start=False, stop=True)
                            nc.tensor.matmul(gt[0:97, 1, s, :], W("I97"),
                                             zinB[0:97, s, :],
                                             start=True, stop=False)
                            nc.tensor.matmul(gt[0:97, 1, s, :], W("whB"),
                                             HT[0:67, s, 0:T],
                                             start=False, stop=True)
                        nc.scalar.activation(SGt[0:33, 0:2, 0:HB, 1:SEG],
                                             gt[0:33, 0:2, :, :], AF.Sigmoid)
                        nc.scalar.activation(SIG, gt[64:97, 0, :, :],
                                             AF.Sigmoid)
                        nc.scalar.activation(TIA, gt[64:97, 1, :, :], AF.Tanh)
                    nc.vector.tensor_mul(S2T[0:33, 0:HB, 1:SEG], SIG, TIA)
                    nc.vector.tensor_tensor_scan(
                        ct_flat, sfg_flat, s2_flat, 0.0, ALU.mult, ALU.add)
                    nc.scalar.activation(TC, CT[0:33, 0:HB, 1:SEG], AF.Tanh)
                    nc.vector.tensor_mul(HL, TC, SGt[0:33, 1, 0:HB, 1:SEG])

                    # CfC layers
                    for lay in range(3):
                        with tc.tile_pool(name="pl", bufs=1,
                                          space="PSUM") as pl:
                            lt = pl.tile([82, HB, T], F32, name="lt")
                            for s in range(HB):
                                if lay == 0:
                                    nc.tensor.matmul(
                                        lt[0:82, s, :], W("I82"),
                                        zinC[0:82, s, :],
                                        start=True, stop=False)
                                    nc.tensor.matmul(
                                        lt[0:82, s, :], W("W0r"),
                                        HL[0:18, s, :],
                                        start=False, stop=True)
                                elif lay == 1:
                                    nc.tensor.matmul(
                                        lt[0:76, s, :], W("P1h0"),
                                        HT[0:18, s, 1:SEG],
                                        start=True, stop=False)
                                    nc.tensor.matmul(
                                        lt[0:76, s, :], W("P1h"),
                                        HL[0:33, s, :],
                                        start=False, stop=True)
                                else:
                                    nc.tensor.matmul(
                                        lt[0:67, s, :], W("P2h1"),
                                        HT[32:44, s, 1:SEG],
                                        start=True, stop=False)
                                    nc.tensor.matmul(
                                        lt[0:67, s, :], W("P2h"),
                                        HL[0:33, s, :],
                                        start=False, stop=True)
                            k = (INTER, COMMAND, MOTOR)[lay]
                            bnm = (None, "b1", "b2")[lay]
                            bias = (lambda a, b: W(bnm)[a:b, 0:1]) if bnm \
                                else (lambda a, b: 0.0)
                            nc.scalar.activation(Ff1[0:k, :, :],
                                                 lt[0:k, :, :], AF.Tanh,
                                                 bias=bias(0, k))
                            nc.scalar.activation(Ff2[0:k, :, :],
                                                 lt[32:32 + k, :, :], AF.Tanh,
                                                 bias=bias(32, 32 + k))
                            nc.scalar.activation(Fti[0:k, :, :],
                                                 lt[64:64 + k, :, :],
                                                 AF.Sigmoid,
                                                 bias=bias(64, 64 + k))
                        nc.vector.tensor_sub(Dg[0:k, :, :], Ff2[0:k, :, :],
                                             Ff1[0:k, :, :])
                        nc.vector.tensor_mul(Gg[0:k, :, :], Fti[0:k, :, :],
                                             Dg[0:k, :, :])
                        hrow = (0, 32, 64)[lay]
                        nc.vector.tensor_add(
                            HT[hrow:hrow + k, 0:HB, 1:SEG],
                            Ff1[0:k, :, :], Gg[0:k, :, :])

                # ---- output: h2 trajectory lives at HT[64:67] ----
                nc.sync.dma_start(
                    out=out_d[:, half * HB:(half + 1) * HB, :],
                    in_=HT[64:67, 0:HB, 1:SEG])
    nc.compile()
    return nc


def host_prep(inputs, T=T_FULL):
    x = np.asarray(inputs["x"], np.float32)
    w = prep_weights(inputs)
    in_maps = []
    for i in range(N_CORES):
        xs = x[i * BS:(i + 1) * BS, :T, :]                  # (BS, T, 512)
        xt = np.ascontiguousarray(
            xs.transpose(2, 0, 1).reshape(IN_DIM, BS * T))  # (512, b*T+t)
        m = {"xt": xt}
        m.update(w)
        in_maps.append(m)
    return in_maps


def gather_output(results, T=T_FULL):
    outs = []
    for i in range(N_CORES):
        o = np.asarray(results[i]["out"])                   # (3, BS, T)
        outs.append(o.transpose(1, 2, 0))                   # (BS, T, 3)
    return np.concatenate(outs, axis=0)


_PROGRAM_CACHE = {}


def kernel(**inputs):
    T = T_FULL
    if T not in _PROGRAM_CACHE:
        _PROGRAM_CACHE[T] = build_program(T)
    nc = _PROGRAM_CACHE[T]
    in_maps = host_prep(inputs, T)
    res = run_bass_kernel_spmd(nc, in_maps, list(range(N_CORES)))
    return gather_output(res.results, T)


# revision 9
# speedup vs baseline: 16.6052x; 8.9012x over previous
"""Trainium2 Bass kernel for the DinMod LSTM+CfC (NCP) recurrent network.

Parallel-in-time Picard iteration. The graded execute path costs ~45us
per emitted instruction (fully serialized, size-independent), so instead
of an exact T=512 sequential scan (~16k instructions), we iterate the
whole trajectory: guess h[t]=0, then each sweep recomputes all T steps
with BATCHED instructions (matmuls/ACT/DVE over 4 sequences x 512 steps
at once). The LSTM c-recurrence is linear given the gates, so ONE
tensor_tensor_scan instruction solves it exactly along the time axis per
sweep. The step map is a strong contraction (~10x error reduction per
sweep, measured on the actual weights): 6 sweeps reach ~3e-6 relative
error in fp64 (tolerance is 2e-2).

Per core: 8 sequences (batch elems), processed as 2 independent halves
of 4 (SBUF budget). Column layout per half: 4 segments of 513 columns
(1 zero pad + 512 steps); col(s, t) = s*513 + 1 + t. Trajectory tiles
read at offset 0 give h[t-1] (shifted), offset 1 gives h[t]; the pad
column doubles as the zero initial state and as the scan reset (the
sfg multiplier and S2 addend are 0 there, so the running c state resets
across segment boundaries in the single flattened scan).

Per sweep per half (~65 instructions):
  gates: 4 segs x (inject zinA | whA@h | inject zinB | whB@h) = 16 mm
         -> sigmoid(fg+1)/sigmoid(og) (1 ACT over both groups),
            sigmoid(ig), tanh(ia) [+1 folded into the zinA bias]
  c:     S2 = sig*tia (DVE), c = tensor_tensor_scan(sfg, S2),
         tanh(c) (ACT), h_lstm = tc*sog (DVE)
  CfC l: 2 mm/seg (inject or h0/h1-part + recurrent part), 3 ACT
         (tanh f1, tanh f2, sigmoid ti), 3 DVE (D=f2-f1, G=ti*D,
         h_l = f1+G -> written time-shifted into the H tile)

Verifier constraints honored: all access patterns start at partition
0/32/64/96; dual-SBUF vector operands have equal base partitions
(mixed SBUF/PSUM exempt); PSUM accumulation groups are start/stop pairs
within one bank.
"""

import numpy as np

import concourse.bass as bass
import concourse.mybir as mybir
from concourse import bacc
from concourse.tile import TileContext
from concourse.bass_utils import run_bass_kernel_spmd

IN_DIM, LATENT = 512, 256
INTER, COMMAND, MOTOR = 18, 12, 3
STATE = INTER + COMMAND + MOTOR  # 33
B, T_FULL, N_CORES = 64, 512, 8
BS = B // N_CORES  # 8 sequences per core
HB = 4             # sequences per half
SEG = T_FULL + 1   # 513 padded columns per sequence
NH = HB * SEG      # 2052 columns per half
NSWEEPS = 4

F32 = mybir.dt.float32
AF = mybir.ActivationFunctionType
ALU = mybir.AluOpType

# ---------------------------------------------------------------------------
# Weight blob: every lhsT/bias lives at rows 0:r, cols off:off+c of a single
# [128, BW] tensor (one DMA; base-0 quadrant-legal slices).
# ---------------------------------------------------------------------------
# (name, rows, cols, base_row): base_row shifts the block down so the
# lhsT slice's base partition matches its rhs (matmul requires equality).
_BLOCKS = [
    ("I97", 97, 97, 0), ("I82", 82, 82, 0),
    ("whA", 67, 97, 0), ("whB", 67, 97, 0),
    ("W0r", 18, 82, 0),
    ("P1h0", 18, 76, 0), ("P1h", 33, 76, 0),
    ("P2h1", 12, 67, 32), ("P2h", 33, 67, 0),
    ("bA", 97, 1, 0), ("bB", 97, 1, 0), ("bC", 82, 1, 0),
    ("b1", 76, 1, 0), ("b2", 67, 1, 0),
] + [(f"pa{g}{k}", 128, 97 if g in "AB" else 82, 0)
     for g in "ABC" for k in range(4)]

_OFFS = {}
_BW = 0
for _nm, _r, _c, _b in _BLOCKS:
    _OFFS[_nm] = (_r, _c, _BW, _b)
    _BW += _c

# HT (h trajectory) row layout: h0@0:18, h1@32:44, h2@64:67 (quadrant-legal
# write starts for the three per-layer h writers); unused rows stay 0.
_HTROW = [j for j in range(18)] + [32 + j for j in range(12)] + [64 + j for j in range(3)]


def prep_weights(inp):
    g = {k: np.asarray(v, np.float64) for k, v in inp.items()}
    Wf, bf = g["fc1_w"], g["fc1_b"]            # (256,512), (256,)
    wi, bi, wh = g["lstm_wi"], g["lstm_bi"], g["lstm_wh"]
    ia, ig, fg, og = (slice(0, 33), slice(33, 66),
                      slice(66, 99), slice(99, 132))

    blob = np.zeros((128, _BW), np.float64)

    def put(nm, m):
        r, c, off, b = _OFFS[nm]
        assert m.shape == (r, c), (nm, m.shape, (r, c))
        blob[b:b + r, off:off + c] = m

    put("I97", np.eye(97))
    put("I82", np.eye(82))

    # recurrent gate contractions: out rows (grp A) fg@0:33, ig@64:97;
    # (grp B) og@0:33, ia@64:97. Contract dim = HT rows (h comp j at _HTROW[j]).
    def wh_block(lo_sl, hi_sl):
        m = np.zeros((67, 97))
        for j in range(STATE):
            r = _HTROW[j]
            m[r, 0:33] = wh[lo_sl, j]
            m[r, 64:97] = wh[hi_sl, j]
        return m

    put("whA", wh_block(fg, ig))
    put("whB", wh_block(og, ia))

    # CfC layer weights (masked), ti = sigmoid((ta+tb) @ xc + (tab+tbb))
    w1m, w2m, wab, bab = [], [], [], []
    for l in range(3):
        w1m.append(g[f"ff1w{l}"] * g[f"mask{l}"])
        w2m.append(g[f"ff2w{l}"] * g[f"mask{l}"])
        wab.append(g[f"taw{l}"] + g[f"tbw{l}"])
        bab.append(g[f"tab{l}"] + g[f"tbb{l}"])

    # layer 0 recurrent part: contracts HL[0:18] (= hs0); out rows
    # f1@0:18, f2@32:50, ti@64:82
    m = np.zeros((18, 82))
    for j in range(INTER):
        m[j, 0:18] = w1m[0][:, LATENT + j]
        m[j, 32:50] = w2m[0][:, LATENT + j]
        m[j, 64:82] = wab[0][:, LATENT + j]
    put("W0r", m)

    # layer 1: xc = [h0(18), hs1(12)]; out rows f1@0:12, f2@32:44, ti@64:76
    m = np.zeros((18, 76))
    for j in range(INTER):
        m[j, 0:12] = w1m[1][:, j]
        m[j, 32:44] = w2m[1][:, j]
        m[j, 64:76] = wab[1][:, j]
    put("P1h0", m)
    m = np.zeros((33, 76))
    for j in range(COMMAND):
        m[INTER + j, 0:12] = w1m[1][:, INTER + j]
        m[INTER + j, 32:44] = w2m[1][:, INTER + j]
        m[INTER + j, 64:76] = wab[1][:, INTER + j]
    put("P1h", m)
    bs1 = np.zeros((76, 1))
    bs1[0:12, 0] = g["ff1b1"]
    bs1[32:44, 0] = g["ff2b1"]
    bs1[64:76, 0] = bab[1]
    put("b1", bs1)

    # layer 2: xc = [h1(12), hs2(3)]; out rows f1@0:3, f2@32:35, ti@64:67
    m = np.zeros((12, 67))
    for j in range(COMMAND):
        m[j, 0:3] = w1m[2][:, j]
        m[j, 32:35] = w2m[2][:, j]
        m[j, 64:67] = wab[2][:, j]
    put("P2h1", m)
    m = np.zeros((33, 67))
    for j in range(MOTOR):
        m[30 + j, 0:3] = w1m[2][:, COMMAND + j]
        m[30 + j, 32:35] = w2m[2][:, COMMAND + j]
        m[30 + j, 64:67] = wab[2][:, COMMAND + j]
    put("P2h", m)
    bs2 = np.zeros((67, 1))
    bs2[0:3, 0] = g["ff1b2"]
    bs2[32:35, 0] = g["ff2b2"]
    bs2[64:67, 0] = bab[2]
    put("b2", bs2)

    # phase A composed input projections (fc1 folded in); +1.0 on the fg
    # bias so the sigmoid ACT needs no extra bias.
    WA = np.zeros((97, IN_DIM)); bA = np.zeros((97, 1))
    WA[0:33] = wi[fg] @ Wf
    bA[0:33, 0] = wi[fg] @ bf + bi[fg] + 1.0
    WA[64:97] = wi[ig] @ Wf
    bA[64:97, 0] = wi[ig] @ bf + bi[ig]
    WB = np.zeros((97, IN_DIM)); bB = np.zeros((97, 1))
    WB[0:33] = wi[og] @ Wf
    bB[0:33, 0] = wi[og] @ bf + bi[og]
    WB[64:97] = wi[ia] @ Wf
    bB[64:97, 0] = wi[ia] @ bf + bi[ia]
    WC = np.zeros((82, IN_DIM)); bC = np.zeros((82, 1))
    WC[0:18] = w1m[0][:, 0:LATENT] @ Wf
    bC[0:18, 0] = w1m[0][:, 0:LATENT] @ bf + g["ff1b0"]
    WC[32:50] = w2m[0][:, 0:LATENT] @ Wf
    bC[32:50, 0] = w2m[0][:, 0:LATENT] @ bf + g["ff2b0"]
    WC[64:82] = wab[0][:, 0:LATENT] @ Wf
    bC[64:82, 0] = wab[0][:, 0:LATENT] @ bf + bab[0]
    put("bA", bA)
    put("bB", bB)
    put("bC", bC)
    for gname, W in (("A", WA), ("B", WB), ("C", WC)):
        for k in range(4):
            put(f"pa{gname}{k}",
                np.ascontiguousarray(W[:, 128 * k:128 * (k + 1)].T))

    return {"wblob": blob.astype(np.float32)}


def build_program(T=T_FULL, opts=()):
    assert T == T_FULL
    opts = set(opts)
    sweep_reps = 1
    for o in opts:
        if isinstance(o, str) and o.startswith("reps"):
            sweep_reps = int(o[4:])

    nc = bacc.Bacc("TRN2")
    xt_d = nc.dram_tensor("xt", [IN_DIM, BS * T], F32, kind="ExternalInput")
    wb_d = nc.dram_tensor("wblob", [128, _BW], F32, kind="ExternalInput")
    out_d = nc.dram_tensor("out", [MOTOR, BS, T], F32, kind="ExternalOutput")

    with TileContext(nc) as tc:
        with tc.tile_pool(name="wpool", bufs=1) as wp, \
             tc.tile_pool(name="data", bufs=1) as dp:
            wb = wp.tile([128, _BW], F32, name="wb")
            nc.sync.dma_start(out=wb, in_=wb_d[:, :])

            def W(nm):
                r, c, off, b = _OFFS[nm]
                return wb[b:b + r, off:off + c]

            # persistent per-half tiles (reused by both halves; only the
            # pad columns must stay zero, and nothing ever writes them)
            zinA = dp.tile([97, HB, T], F32, name="zinA")
            zinB = dp.tile([97, HB, T], F32, name="zinB")
            zinC = dp.tile([82, HB, T], F32, name="zinC")
            SGt = dp.tile([33, 2, HB, SEG], F32, name="SGt")  # sfg | sog
            SIG = dp.tile([33, HB, T], F32, name="SIG")
            TIA = dp.tile([33, HB, T], F32, name="TIA")
            S2T = dp.tile([33, HB, SEG], F32, name="S2T")
            CT = dp.tile([33, HB, SEG], F32, name="CT")
            TC = dp.tile([33, HB, T], F32, name="TC")
            HT = dp.tile([67, HB, SEG], F32, name="HT")
            HL = dp.tile([33, HB, T], F32, name="HL")
            Ff1 = dp.tile([18, HB, T], F32, name="Ff1")
            Ff2 = dp.tile([18, HB, T], F32, name="Ff2")
            Fti = dp.tile([18, HB, T], F32, name="Fti")
            Dg = dp.tile([18, HB, T], F32, name="Dg")
            Gg = dp.tile([18, HB, T], F32, name="Gg")

            nc.vector.memset(SGt, 0.0)
            nc.vector.memset(S2T, 0.0)

            sfg_flat = SGt.rearrange("p g s c -> p g (s c)")[0:33, 0, 0:NH]
            s2_flat = S2T.rearrange("p s c -> p (s c)")
            ct_flat = CT.rearrange("p s c -> p (s c)")

            xt_r = xt_d.rearrange("(c p) n -> p c n", p=128)

            for half in range(2):
                hc0 = half * HB * T  # first input column of this half
                # sweep 1 exploits HT == 0 (gate matmuls skipped entirely)
                nc.vector.memset(HT, 0.0)
                with tc.tile_pool(name="xp", bufs=1) as xp:
                    xt_sb = xp.tile([128, 4, HB * T], F32, name="xt_sb")
                    nc.sync.dma_start(
                        out=xt_sb, in_=xt_r[:, :, hc0:hc0 + HB * T])

                    # ---- phase A: input projections -> zinA/zinB/zinC ----
                    with tc.tile_pool(name="pa", bufs=1, space="PSUM") as pa:
                        pg = pa.tile([97, 2, HB, T], F32, name="pg")
                        for gi, gname in ((0, "A"), (1, "B")):
                            for s in range(HB):
                                for k in range(4):
                                    nc.tensor.matmul(
                                        pg[0:97, gi, s, :],
                                        W(f"pa{gname}{k}")[:, 0:97],
                                        xt_sb[:, k, s * T:(s + 1) * T],
                                        start=(k == 0), stop=(k == 3))
                        nc.scalar.activation(zinA, pg[0:97, 0, :, :],
                                             AF.Identity, bias=W("bA")[:, 0:1])
                        nc.scalar.activation(zinB, pg[0:97, 1, :, :],
                                             AF.Identity, bias=W("bB")[:, 0:1])
                    with tc.tile_pool(name="pc", bufs=1, space="PSUM") as pc:
                        pgc = pc.tile([82, HB, T], F32, name="pgc")
                        for s in range(HB):
                            for k in range(4):
                                nc.tensor.matmul(
                                    pgc[0:82, s, :], W(f"paC{k}")[:, 0:82],
                                    xt_sb[:, k, s * T:(s + 1) * T],
                                    start=(k == 0), stop=(k == 3))
                        nc.scalar.activation(zinC, pgc, AF.Identity,
                                             bias=W("bC")[:, 0:1])

                # ---- Picard sweeps ----
                for sw in range(NSWEEPS * sweep_reps):
                    # gates
                    if sw == 0:
                        # HT == 0: gate preacts are just zinA/zinB
                        nc.scalar.activation(SGt[0:33, 0, 0:HB, 1:SEG],
                                             zinA[0:33, :, :], AF.Sigmoid)
                        nc.scalar.activation(SGt[0:33, 1, 0:HB, 1:SEG],
                                             zinB[0:33, :, :], AF.Sigmoid)
                        nc.scalar.activation(SIG, zinA[64:97, :, :],
                                             AF.Sigmoid)
                        nc.scalar.activation(TIA, zinB[64:97, :, :], AF.Tanh)
                    else:
                        with tc.tile_pool(name="pq", bufs=1,
                                          space="PSUM") as pq:
                            gt = pq.tile([97, 2, HB, T], F32, name="gt")
                            for s in range(HB):
                                nc.tensor.matmul(gt[0:97, 0, s, :], W("I97"),
                                                 zinA[0:97, s, :],
                                                 start=True, stop=False)
                                nc.tensor.matmul(gt[0:97, 0, s, :], W("whA"),
                                                 HT[0:67, s, 0:T],
                                                 start=False, stop=True)
                                nc.tensor.matmul(gt[0:97, 1, s, :], W("I97"),
                                                 zinB[0:97, s, :],
                                                 start=True, stop=False)
                                nc.tensor.matmul(gt[0:97, 1, s, :], W("whB"),
                                                 HT[0:67, s, 0:T],
                                                 start=False, stop=True)
                            nc.scalar.activation(SGt[0:33, 0:2, 0:HB, 1:SEG],
                                                 gt[0:33, 0:2, :, :],
                                                 AF.Sigmoid)
                            nc.scalar.activation(SIG, gt[64:97, 0, :, :],
                                                 AF.Sigmoid)
                            nc.scalar.activation(TIA, gt[64:97, 1, :, :],
                                                 AF.Tanh)
                    nc.vector.tensor_mul(S2T[0:33, 0:HB, 1:SEG], SIG, TIA)
                    nc.vector.tensor_tensor_scan(
                        ct_flat, sfg_flat, s2_flat, 0.0, ALU.mult, ALU.add)
                    nc.scalar.activation(TC, CT[0:33, 0:HB, 1:SEG], AF.Tanh)
                    nc.vector.tensor_mul(HL, TC, SGt[0:33, 1, 0:HB, 1:SEG])

                    # CfC layers
                    for lay in range(3):
                        with tc.tile_pool(name="pl", bufs=1,
                                          space="PSUM") as pl:
                            lt = pl.tile([82, HB, T], F32, name="lt")
                            for s in range(HB):
                                if lay == 0:
                                    nc.tensor.matmul(
                                        lt[0:82, s, :], W("I82"),
                                        zinC[0:82, s, :],
                                        start=True, stop=False)
                                    nc.tensor.matmul(
                                        lt[0:82, s, :], W("W0r"),
                                        HL[0:18, s, :],
                                        start=False, stop=True)
                                elif lay == 1:
                                    nc.tensor.matmul(
                                        lt[0:76, s, :], W("P1h0"),
                                        HT[0:18, s, 1:SEG],
                                        start=True, stop=False)
                                    nc.tensor.matmul(
                                        lt[0:76, s, :], W("P1h"),
                                        HL[0:33, s, :],
                                        start=False, stop=True)
                                else:
                                    nc.tensor.matmul(
                                        lt[0:67, s, :], W("P2h1"),
                                        HT[32:44, s, 1:SEG],
                                        start=True, stop=False)
                                    nc.tensor.matmul(
                                        lt[0:67, s, :], W("P2h"),
                                        HL[0:33, s, :],
                                        start=False, stop=True)
                            k = (INTER, COMMAND, MOTOR)[lay]
                            bnm = (None, "b1", "b2")[lay]
                            bias = (lambda a, b: W(bnm)[a:b, 0:1]) if bnm \
                                else (lambda a, b: 0.0)
                            nc.scalar.activation(Ff1[0:k, :, :],
                                                 lt[0:k, :, :], AF.Tanh,
                                                 bias=bias(0, k))
                            nc.scalar.activation(Ff2[0:k, :, :],
                                                 lt[32:32 + k, :, :], AF.Tanh,
                                                 bias=bias(32, 32 + k))
                            nc.scalar.activation(Fti[0:k, :, :],
                                                 lt[64:64 + k, :, :],
                                                 AF.Sigmoid,
                                                 bias=bias(64, 64 + k))
                        nc.vector.tensor_sub(Dg[0:k, :, :], Ff2[0:k, :, :],
                                             Ff1[0:k, :, :])
                        nc.vector.tensor_mul(Gg[0:k, :, :], Fti[0:k, :, :],
                                             Dg[0:k, :, :])
                        hrow = (0, 32, 64)[lay]
                        nc.vector.tensor_add(
                            HT[hrow:hrow + k, 0:HB, 1:SEG],
                            Ff1[0:k, :, :], Gg[0:k, :, :])

                # ---- output: h2 trajectory lives at HT[64:67] ----
                nc.sync.dma_start(
                    out=out_d[:, half * HB:(half + 1) * HB, :],
                    in_=HT[64:67, 0:HB, 1:SEG])
    nc.compile()
    return nc


def host_prep(inputs, T=T_FULL):
    x = np.asarray(inputs["x"], np.float32)
    w = prep_weights(inputs)
    in_maps = []
    for i in range(N_CORES):
        xs = x[i * BS:(i + 1) * BS, :T, :]                  # (BS, T, 512)
        xt = np.ascontiguousarray(
            xs.transpose(2, 0, 1).reshape(IN_DIM, BS * T))  # (512, b*T+t)
        m = {"xt": xt}
        m.update(w)
        in_maps.append(m)
    return in_maps


def gather_output(results, T=T_FULL):
    outs = []
    for i in range(N_CORES):
        o = np.asarray(results[i]["out"])                   # (3, BS, T)
        outs.append(o.transpose(1, 2, 0))                   # (BS, T, 3)
    return np.concatenate(outs, axis=0)


_PROGRAM_CACHE = {}


def kernel(**inputs):
    T = T_FULL
    if T not in _PROGRAM_CACHE:
        _PROGRAM_CACHE[T] = build_program(T)
    nc = _PROGRAM_CACHE[T]
    in_maps = host_prep(inputs, T)
    res = run_bass_kernel_spmd(nc, in_maps, list(range(N_CORES)))
    return gather_output(res.results, T)
